# revision 1
# baseline (speedup 1.0000x reference)
"""Fused attention block (LGHIFusion) for Trainium2, 8-core tensor-parallel.

Math (per reference):
  Q = low  @ W_Q.T + b_Q ; K = low @ W_K.T + b_K ; V = high @ W_V.T + b_V
  attn = softmax(Q K^T / sqrt(dh)) ; ctx = attn @ V
  Z = ctx @ W_O.T + b_O ; out = low + sigmoid(gamma) * Z

Sharding: tensor-parallel over heads. 16 heads / 8 cores = 2 heads/core.
Each core computes QT/KT/VT for its 128 output dims, per-head attention
with scores kept TRANSPOSED ([k, q] layout) so softmax denominators come
free from an appended ones-column in V (no PE transposes of P needed),
then its partial Z = ctx @ W_O[:, shard].T (full 1024 output dims).
Host sums the 8 fp16 partials and applies residual + beta*b_O.

All matmuls run in bf16 (full PE rate, FWL weight loads, half DMA);
the beta=sigmoid(-5)~0.0067 gate damps kernel error by ~150x in the
final output, so bf16/fp16-partials error is small end to end.
"""

import numpy as np

try:
    import concourse.bass as bass
except ImportError:  # pragma: no cover
    import sys

    sys.path.insert(0, "/opt/trn_rl_repo")
    import concourse.bass as bass

import concourse.mybir as mybir
from concourse.bass_utils import run_bass_kernel_spmd
from concourse.masks import make_identity
from concourse.tile import TileContext

dt = mybir.dt
F32, BF16, F16 = dt.float32, dt.bfloat16, dt.float16
AF = mybir.ActivationFunctionType

B, S, D = 2, 2048, 1024
H, DH = 16, 64
T = B * S            # 4096 tokens
NCORES = 8
HPC = H // NCORES    # 2 heads per core
OPC = HPC * DH       # 128 out dims per core
VW = DH + 1          # V columns + ones column = 65
KT_N = S // 128      # 16 k-tiles per batch
NKT = T // 128       # 32 global token tiles
PCH = 512            # projection token-chunk size
QC = 1024            # q-chunk for attention


def _build_nc():
    nc = bass.Bass("TRN2", target_bir_lowering=False, debug=False,
                   num_devices=NCORES)

    xt_lo = nc.dram_tensor("xt_lo", [D, T], BF16, kind="ExternalInput").ap()
    xt_hi = nc.dram_tensor("xt_hi", [D, T], BF16, kind="ExternalInput").ap()
    wq_t = nc.dram_tensor("wq_t", [D, OPC], BF16, kind="ExternalInput").ap()
    wk_t = nc.dram_tensor("wk_t", [D, OPC], BF16, kind="ExternalInput").ap()
    wv_t = nc.dram_tensor("wv_t", [D, OPC], BF16, kind="ExternalInput").ap()
    wo_t = nc.dram_tensor("wo_t", [OPC, D], BF16, kind="ExternalInput").ap()
    bq_d = nc.dram_tensor("bq", [1, OPC], BF16, kind="ExternalInput").ap()
    bk_d = nc.dram_tensor("bk", [1, OPC], BF16, kind="ExternalInput").ap()
    bv_d = nc.dram_tensor("bv", [1, OPC], BF16, kind="ExternalInput").ap()
    z_out = nc.dram_tensor("z_out", [T, D], F16, kind="ExternalOutput").ap()

    with TileContext(nc) as tc:
        with (
            tc.tile_pool(name="const", bufs=1) as const,
            tc.tile_pool(name="w", bufs=1) as wpool,
            tc.tile_pool(name="x", bufs=2) as xpool,
            tc.tile_pool(name="acts", bufs=1) as actpool,
            tc.tile_pool(name="vone", bufs=1) as vpool,
            tc.tile_pool(name="pt", bufs=3) as ptpool,
            tc.tile_pool(name="ctxn", bufs=2) as cxpool,
            tc.tile_pool(name="z16", bufs=3) as zpool,
            tc.tile_pool(name="r", bufs=2) as rpool,
            tc.tile_pool(name="ps", bufs=2, space="PSUM") as pp,
            tc.tile_pool(name="pc", bufs=1, space="PSUM") as pc,
        ):
            ident = const.tile([128, 128], BF16)
            make_identity(nc, ident[:])

            wq = wpool.tile([128, D], BF16, tag="wq")
            wk = wpool.tile([128, D], BF16, tag="wk")
            wv = wpool.tile([128, D], BF16, tag="wv")
            wo = wpool.tile([128, D], BF16, tag="wo")
            for k in range(D // 128):
                nc.sync.dma_start(wq[:, 128 * k:128 * (k + 1)],
                                  wq_t[128 * k:128 * (k + 1), :])
                nc.sync.dma_start(wk[:, 128 * k:128 * (k + 1)],
                                  wk_t[128 * k:128 * (k + 1), :])
                nc.sync.dma_start(wv[:, 128 * k:128 * (k + 1)],
                                  wv_t[128 * k:128 * (k + 1), :])
            nc.sync.dma_start(wo[:], wo_t[:, :])
            bq = const.tile([1, OPC], BF16, tag="bq")
            bk = const.tile([1, OPC], BF16, tag="bk")
            bv = const.tile([1, OPC], BF16, tag="bv")
            nc.sync.dma_start(bq[:], bq_d[:, :])
            nc.sync.dma_start(bk[:], bk_d[:, :])
            nc.sync.dma_start(bv[:], bv_d[:, :])
            ones_p = const.tile([1, PCH], BF16, tag="ones_p")
            nc.vector.memset(ones_p[:], 1.0)
            ones64 = const.tile([1, DH], F32, tag="ones64")
            nc.vector.memset(ones64[:], 1.0)

            # Persistent activations: [128 outdims, token] transposed layout.
            qt = actpool.tile([128, T], BF16, tag="qt")
            kts = actpool.tile([128, T], BF16, tag="kt")
            vts = actpool.tile([128, T], BF16, tag="vt")
            # V in [k, dh] layout + ones column per (ktile, head).
            vone = vpool.tile([128, NKT * HPC * VW], BF16)
            nc.vector.memset(vone[:], 1.0)

            # ---- Phase B: projections (QT/KT/VT), streamed over tokens ----
            nd = D // 128
            for tch in range(T // PCH):
                t0 = tch * PCH
                xlo = xpool.tile([128, nd * PCH], BF16, tag="xlo")
                xhi = xpool.tile([128, nd * PCH], BF16, tag="xhi")
                for k in range(nd):
                    nc.sync.dma_start(xlo[:, PCH * k:PCH * (k + 1)],
                                      xt_lo[128 * k:128 * (k + 1), t0:t0 + PCH])
                    nc.sync.dma_start(xhi[:, PCH * k:PCH * (k + 1)],
                                      xt_hi[128 * k:128 * (k + 1), t0:t0 + PCH])
                for wmat, bias, dest, src in (
                    (wq, bq, qt, xlo),
                    (wk, bk, kts, xlo),
                    (wv, bv, vts, xhi),
                ):
                    ps = pp.tile([128, PCH], F32, tag="s")
                    for k in range(nd):
                        nc.tensor.matmul(
                            ps[:],
                            lhsT=wmat[:, 128 * k:128 * (k + 1)],
                            rhs=src[:, PCH * k:PCH * (k + 1)],
                            start=(k == 0), stop=False)
                    nc.tensor.matmul(ps[:], lhsT=bias[:], rhs=ones_p[:],
                                     start=False, stop=True)
                    nc.vector.tensor_copy(dest[:, t0:t0 + PCH], ps[:])

            # ---- Phase C: V -> [k, dh] via PE transpose, into vone ----
            for g in range(NKT):
                pt_ps = pc.tile([128, 128], BF16, tag="c")
                nc.tensor.transpose(pt_ps[:], vts[:, 128 * g:128 * (g + 1)],
                                    ident[:])
                for h in range(HPC):
                    base = (g * HPC + h) * VW
                    nc.vector.tensor_copy(vone[:, base:base + DH],
                                          pt_ps[:, DH * h:DH * (h + 1)])

            # ---- Phase D: attention, scores transposed [k, q] ----
            for b in range(B):
                ctxn = cxpool.tile([128, S], BF16)
                for h in range(HPC):
                    hp = DH * h
                    for qc in range(S // QC):
                        q0 = b * S + qc * QC
                        ps_c = pc.tile([VW, QC], F32, tag="c")
                        for kt in range(KT_N):
                            g = b * KT_N + kt
                            ps_s = pp.tile([128, QC], F32, tag="s")
                            for hf in range(QC // 512):
                                nc.tensor.matmul(
                                    ps_s[:, 512 * hf:512 * (hf + 1)],
                                    lhsT=kts[hp:hp + DH,
                                                   128 * g:128 * (g + 1)],
                                    rhs=qt[hp:hp + DH,
                                                 q0 + 512 * hf:
                                                 q0 + 512 * (hf + 1)],
                                    start=True, stop=True)
                            pt = ptpool.tile([128, QC], BF16)
                            nc.scalar.activation(pt[:], ps_s[:], AF.Exp,
                                                 scale=0.125)
                            vbase = (g * HPC + h) * VW
                            for hf in range(QC // 512):
                                nc.tensor.matmul(
                                    ps_c[:, 512 * hf:512 * (hf + 1)],
                                    lhsT=vone[:, vbase:vbase + VW],
                                    rhs=pt[:, 512 * hf:512 * (hf + 1)],
                                    start=(kt == 0), stop=(kt == KT_N - 1))
                        recip = rpool.tile([1, QC], F32, tag="recip")
                        nc.vector.reciprocal(recip[:], ps_c[DH:DH + 1, :])
                        ps_bc = pc.tile([DH, QC], F32, tag="bc")
                        for hf in range(QC // 512):
                            nc.tensor.matmul(
                                ps_bc[:, 512 * hf:512 * (hf + 1)],
                                lhsT=ones64[:],
                                rhs=recip[:, 512 * hf:512 * (hf + 1)],
                                start=True, stop=True)
                        bc_sb = rpool.tile([DH, QC], F32, tag="bc")
                        nc.vector.tensor_copy(bc_sb[:], ps_bc[:])
                        nc.vector.tensor_mul(
                            ctxn[hp:hp + DH, qc * QC:(qc + 1) * QC],
                            ps_c[0:DH, :], bc_sb[:])

                # ---- Phase E: partial Z = ctxN.T @ W_O_shard.T ----
                for qt_i in range(S // 128):
                    ps_z = pp.tile([128, D], F32, tag="s")
                    for hf in range(D // 512):
                        nc.tensor.matmul(
                            ps_z[:, 512 * hf:512 * (hf + 1)],
                            lhsT=ctxn[:, 128 * qt_i:128 * (qt_i + 1)],
                            rhs=wo[:, 512 * hf:512 * (hf + 1)],
                            start=True, stop=True)
                    z16 = zpool.tile([128, D], F16)
                    nc.vector.tensor_copy(z16[:], ps_z[:])
                    r0 = b * S + 128 * qt_i
                    nc.sync.dma_start(z_out[r0:r0 + 128, :], z16[:])

    _split_waits(nc)
    return nc


def _split_waits(nc):
    """This walrus build accepts only one sync-wait per instruction.
    Move extra waits onto same-engine NoOps inserted just before each
    offender (engine program order preserves the gating)."""
    for f in nc.m.functions:
        for blk in f.blocks:
            new_insts = []
            for inst in blk.instructions:
                si = inst.sync_info
                if si is not None and si.on_wait and len(si.on_wait) > 1:
                    waits = list(si.on_wait)
                    for w in waits[:-1]:
                        nop = mybir.InstNoOp(
                            name=nc.get_next_instruction_name(),
                            sync_info=mybir.SyncInfo(on_wait=[w],
                                                     on_update=[]),
                            bass_nofuse=True,
                            engine=inst.engine,
                        )
                        new_insts.append(nop)
                    si.on_wait = [waits[-1]]
                new_insts.append(inst)
            blk.instructions[:] = new_insts


_NC_CACHE = None


def _get_nc():
    global _NC_CACHE
    if _NC_CACHE is None:
        _NC_CACHE = _build_nc()
    return _NC_CACHE


def _make_in_maps(inputs):
    low = np.ascontiguousarray(np.asarray(inputs["low_freq"], np.float32))
    high = np.ascontiguousarray(np.asarray(inputs["high_freq"], np.float32))
    W_Q = np.asarray(inputs["W_Q"], np.float32)
    W_K = np.asarray(inputs["W_K"], np.float32)
    W_V = np.asarray(inputs["W_V"], np.float32)
    W_O = np.asarray(inputs["W_O"], np.float32)
    b_Q = np.asarray(inputs["b_Q"], np.float32)
    b_K = np.asarray(inputs["b_K"], np.float32)
    b_V = np.asarray(inputs["b_V"], np.float32)

    import ml_dtypes
    bf16 = ml_dtypes.bfloat16
    xt_lo = np.ascontiguousarray(low.reshape(T, D).T.astype(bf16))
    xt_hi = np.ascontiguousarray(high.reshape(T, D).T.astype(bf16))

    in_maps = []
    for c in range(NCORES):
        sl = slice(OPC * c, OPC * (c + 1))
        in_maps.append({
            "xt_lo": xt_lo,
            "xt_hi": xt_hi,
            "wq_t": np.ascontiguousarray(W_Q[sl, :].T.astype(bf16)),
            "wk_t": np.ascontiguousarray(W_K[sl, :].T.astype(bf16)),
            "wv_t": np.ascontiguousarray(W_V[sl, :].T.astype(bf16)),
            "wo_t": np.ascontiguousarray(W_O[:, sl].T.astype(bf16)),
            "bq": np.ascontiguousarray(b_Q[sl].reshape(1, OPC).astype(bf16)),
            "bk": np.ascontiguousarray(b_K[sl].reshape(1, OPC).astype(bf16)),
            "bv": np.ascontiguousarray(b_V[sl].reshape(1, OPC).astype(bf16)),
        })
    return in_maps


def _run(inputs, trace=False, **kw):
    low = np.ascontiguousarray(np.asarray(inputs["low_freq"], np.float32))
    b_O = np.asarray(inputs["b_O"], np.float32)
    gamma = float(np.asarray(inputs["gamma"], np.float32))
    in_maps = _make_in_maps(inputs)

    nc = _get_nc()
    res = run_bass_kernel_spmd(nc, in_maps, list(range(NCORES)), trace=trace,
                               **kw)

    zsum = np.zeros((T, D), np.float32)
    for r in res.results:
        zsum += r["z_out"].astype(np.float32)
    beta = 1.0 / (1.0 + np.exp(-gamma))
    out = low.reshape(T, D) + beta * (zsum + b_O[None, :])
    return out.reshape(B, S, D), res


def kernel(**inputs):
    out, _ = _run(inputs)
    return out

